# revision 40
# baseline (speedup 1.0000x reference)
"""Multi-head self-attention (RoPE, causal) Bass kernel for 8 TRN2 NeuronCores.

Sharding: tensor-parallel over heads for QKV+attention (2 heads/core),
chunked AllToAll (1 chunk per 512-token q-slab, fp16) overlapped with
attention compute, then token-parallel O-projection (strided 64-token
strips per core).

v2 schedule:
  - attention q-slabs processed in DESCENDING order (qi=3..0) so the
    last-shipped AllToAll chunk is the cheapest slab and the tail is
    normalize + one small collective + one O-projection.
  - attn@V matmuls deferred 2 kt-blocks behind the score matmuls
    (global deque across qi boundaries) so the exp->mask chain never
    stalls the PE.
  - RoPE computed in fp16: Scalar casts PSUM->SBUF, DVE does
    shuffle + 3 wide TTs (2x 16-bit mode).
  - v-transpose copies + casts on the Scalar engine (idle in proj phase).
  - O-projections emitted before the next collective trigger so the
    cumulative CC semaphore never adds false waits.
  - final O-projection os-split with early per-half y DMA.

Layouts (per core):
  qkT:   [128 part = 2 heads x 64 dk, 2 (q|k), t] fp16
  sc:    [128 part = k-tile, 2 heads x 512 q] PSUM f32
  at:    same shape, fp16, exp(sc*0.125), causal-masked via 0/1 TT mult
  v_sb:  [128 part = k-tile tokens, 16 kt, 130] fp16 ([v_h0 | 1 | v_h1 | 1])
  outT:  [65 part = 64 dk + denom row, 2 x 512 q] PSUM f32 (both heads)
  aoT:   [128, t] fp16 normalized attention output
  y:     [128 part = 2 x 64-token strips, 1024] f32 per (unit, pair)
"""

import numpy as np

B, S, D, H, DK = 2, 2048, 1024, 16, 64
NC = 8
THETA = 10000.0

_COMPILED = {}


def _build():
    import concourse.bass as bass
    import concourse.tile as tile
    from concourse import bacc, mybir

    f32 = mybir.dt.float32
    f32r = mybir.dt.float32r
    f16 = mybir.dt.float16
    MUL = mybir.AluOpType.mult
    ADD = mybir.AluOpType.add
    EXP = mybir.ActivationFunctionType.Exp
    COPY = mybir.ActivationFunctionType.Copy

    nc = bacc.Bacc(num_devices=NC)

    xh_d = nc.dram_tensor("xh", [B, 4, 128, 8, 512], f16, kind="ExternalInput")
    wqkv_d = nc.dram_tensor("wqkv", [128, 3, 8, 128], f16, kind="ExternalInput")
    wo_d = nc.dram_tensor("wo", [128, 8, 1024], f16, kind="ExternalInput")
    # csd: [128, 2 (cos|sinm), 2 (q|k dup), S] fp16
    csd_d = nc.dram_tensor("csd", [128, 2, 2, S], f16, kind="ExternalInput")
    cb_d = nc.dram_tensor("cb", [128, 672], f16, kind="ExternalInput")
    onesr_d = nc.dram_tensor("onesr", [1, 64], f32r, kind="ExternalInput")
    y_d = nc.dram_tensor("y", [B, 2, 128, 1024], f32, kind="ExternalOutput")

    SWAP_MASK = [(i ^ 1) for i in range(32)]

    with tile.TileContext(nc) as tc:
        with (
            tc.tile_pool(name="const", bufs=1) as constp,
            tc.tile_pool(name="xtp", bufs=2) as xtp,
            tc.tile_pool(name="qk", bufs=2) as qkp,
            tc.tile_pool(name="vp", bufs=2) as vp,
            tc.tile_pool(name="attn", bufs=3) as attnp,
            tc.tile_pool(name="ao", bufs=2) as aop,
            tc.tile_pool(name="rtmp", bufs=2) as rtmp,
            tc.tile_pool(name="recipp", bufs=2) as recipp,
            tc.tile_pool(name="rbp", bufs=2) as rbp,
            tc.tile_pool(name="gp", bufs=2) as gp,
            tc.tile_pool(name="gp2", bufs=1) as gp2,
            tc.tile_pool(name="yp", bufs=2) as yp,
            tc.tile_pool(name="ps", bufs=1, space="PSUM") as psp,
            tc.tile_pool(name="dram", bufs=1, space="DRAM") as dramp,
        ):
            # ---- constant tiles ----
            wqkv_sb = constp.tile([128, 3, 8, 128], f16)
            wo_sb = constp.tile([128, 8, 1024], f16)
            csd_sb = constp.tile([128, 2, 2, S], f16)
            cb_sb = constp.tile([128, 672], f16)
            onesr_sb = constp.tile([1, 64], f32r)

            mask0 = cb_sb[:, 0:256]
            mask1 = cb_sb[:, 256:512]
            identb = cb_sb[:, 512:640]
            onesb = cb_sb[:, 640:656]

            # critical path: projection weights first (sync queue), split so
            # the first dc matmuls can start before the full tensor lands
            nc.sync.dma_start(wqkv_sb[:, :, 0:2, :], wqkv_d[:, :, 0:2, :])
            nc.sync.dma_start(wqkv_sb[:, :, 2:8, :], wqkv_d[:, :, 2:8, :])

            warm_in = dramp.tile([NC, 64], f32, name="warm_in")
            warm_out = dramp.tile([NC, 64], f32, name="warm_out")
            nc.gpsimd.collective_compute(
                "AllToAll",
                mybir.AluOpType.bypass,
                replica_groups=[list(range(NC))],
                ins=[warm_in.opt()],
                outs=[warm_out.opt()],
            )
            # startup DMA bandwidth is the projection-phase bottleneck:
            # load only what the first tiles need, defer the rest
            # (csd half 2 at u0-tt1, wo at u0-tt3)
            nc.gpsimd.dma_start(cb_sb[:], cb_d[:])
            nc.gpsimd.dma_start(onesr_sb[:], onesr_d[:])
            nc.gpsimd.dma_start(csd_sb[:, :, :, 0:1024], csd_d[:, :, :, 0:1024])

            a2ain = [
                [dramp.tile([NC, 128, 64], f16, name=f"a2ai{u}_{c}") for c in range(4)]
                for u in range(B)
            ]
            a2aout = [
                [dramp.tile([NC, 128, 64], f16, name=f"a2ao{u}_{c}") for c in range(4)]
                for u in range(B)
            ]

            def o_proj_gather(u, pair, tag="g"):
                g = gp.tile([128, 8, 128], f16, tag=tag, name="g")
                for half in range(2):
                    nc.sync.dma_start(
                        g[:, :, half * 64:(half + 1) * 64],
                        a2aout[u][pair * 2 + half].rearrange("j p t -> p j t"),
                    )
                return g

            def o_proj_compute(u, pair, g, y_tag="outT"):
                y_ps = psp.tile([128, 1024], f32, tag=y_tag, bufs=2, name="y_ps")
                y_sb = yp.tile([128, 1024], f32, tag="y", name="y_sb")
                for os_ in range(2):
                    osl = slice(os_ * 512, (os_ + 1) * 512)
                    for src in range(8):
                        nc.tensor.matmul(
                            y_ps[:, osl],
                            g[:, src, :],
                            wo_sb[:, src, osl],
                            start=(src == 0), stop=(src == 7),
                        )
                    nc.vector.tensor_copy(out=y_sb[:, osl], in_=y_ps[:, osl])
                    nc.scalar.dma_start(y_d[u, pair][:, osl], y_sb[:, osl])

            def o_projection(u, pair, y_tag="outT"):
                o_proj_compute(u, pair, o_proj_gather(u, pair), y_tag)

            # deferred normalize+ship state: (u, qi, outT, aoT)
            pending = []

            def emit_normalize(final=False):
                u, qi, outT, aoT = pending.pop()
                qsl = slice(qi * 512, (qi + 1) * 512)
                dentr = recipp.tile([1, 1024], f32r, tag="recip", name="dentr")
                nc.vector.tensor_copy(out=dentr[:], in_=outT[64:65, :])
                rb_ps = psp.tile([128, 1024], f32, tag="sc", bufs=2, name="rb_ps")
                for h in (0, 1):
                    # broadcast denominators to 64 partitions via an
                    # f32r ones-column outer product
                    nc.tensor.matmul(
                        rb_ps[0:64, h * 512:(h + 1) * 512],
                        onesr_sb[:],
                        dentr[0:1, h * 512:(h + 1) * 512],
                        start=True, stop=True,
                    )
                rb_sb = rbp.tile([128, 1024], f32, tag="rb", name="rb_sb")
                nc.vector.reciprocal_approx_fast(out=rb_sb[0:64, :], in_=rb_ps[0:64, :])
                for h in (0, 1):
                    nc.vector.tensor_tensor(
                        out=aoT[h * 64:(h + 1) * 64, qsl],
                        in0=outT[0:64, h * 512:(h + 1) * 512],
                        in1=rb_sb[0:64, h * 512:(h + 1) * 512],
                        op=MUL,
                    )
                # ship this q-slab as its own a2a chunk
                dst = a2ain[u][qi].rearrange("j p t -> p j t")
                src = aoT[:, qsl].rearrange("p (j t) -> p j t", j=8)
                nc.gpsimd.dma_start(dst, src)
                nc.gpsimd.collective_compute(
                    "AllToAll",
                    mybir.AluOpType.bypass,
                    replica_groups=[list(range(NC))],
                    ins=[a2ain[u][qi].opt()],
                    outs=[a2aout[u][qi].opt()],
                )

            for u in range(B):
                # ================= projections + RoPE =================
                qkT = qkp.tile([128, 2, S], f16, tag="qkT", name="qkT")
                v_sb = vp.tile([128, 16, 130], f16, tag="v", name="v_sb")

                for tt in range(4):
                    ts = slice(tt * 512, (tt + 1) * 512)
                    xt_sb = xtp.tile([128, 8, 512], f16, tag="xt", name="xt_sb")
                    if u == 0 and tt == 0:
                        nc.sync.dma_start(xt_sb[:, 0:2, :], xh_d[u, tt][:, 0:2, :])
                        nc.sync.dma_start(xt_sb[:, 2:8, :], xh_d[u, tt][:, 2:8, :])
                    else:
                        nc.sync.dma_start(xt_sb[:], xh_d[u, tt])
                    if u == 0 and tt == 1:
                        nc.gpsimd.dma_start(csd_sb[:, :, :, 1024:2048],
                                            csd_d[:, :, :, 1024:2048])
                    if u == 0 and tt == 3:
                        nc.gpsimd.dma_start(wo_sb[:], wo_d[:])
                    qk_ps = psp.tile([128, 1024], f32, tag="sc", bufs=2, name="qk_ps")
                    v_ps = psp.tile([128, 1024], f32, tag="outT", bufs=2, name="v_ps")
                    for dc in range(8):
                        st = dc == 0
                        sp = dc == 7
                        rhs = xt_sb[:, dc, :]
                        nc.tensor.matmul(qk_ps[:, 0:512], wqkv_sb[:, 0, dc, :], rhs, start=st, stop=sp)
                        nc.tensor.matmul(qk_ps[:, 512:1024], wqkv_sb[:, 1, dc, :], rhs, start=st, stop=sp)
                        nc.tensor.matmul(v_ps[:, 0:512], wqkv_sb[:, 2, dc, :], rhs, start=st, stop=sp)

                    if u > 0 and tt == 0:
                        # prior unit's first O-projection (chunks 0,1 landed
                        # long ago; must precede the c(u-1,3) trigger so the
                        # cumulative cc semaphore adds no false wait), then
                        # ship the prior unit's last chunk. y_ps on the "sc"
                        # ring here: the "outT" ring still holds the prior
                        # unit's un-normalized outT (deadlock otherwise).
                        o_projection(u - 1, 0, y_tag="sc")
                        while pending:
                            emit_normalize()

                    # RoPE in fp16: Scalar casts, DVE rotates
                    qk16 = rtmp.tile([128, 2, 512], f16, tag="qk16", name="qk16")
                    qs16 = rtmp.tile([128, 2, 512], f16, tag="qs16", name="qs16")
                    nc.scalar.activation(
                        out=qk16.rearrange("p a t -> p (a t)"),
                        in_=qk_ps[:], func=COPY)
                    nc.vector.stream_shuffle(
                        qs16.rearrange("p a t -> p (a t)"),
                        qk16.rearrange("p a t -> p (a t)"), SWAP_MASK)
                    nc.vector.tensor_tensor(
                        out=qk16[:], in0=qk16[:], in1=csd_sb[:, 0, :, ts], op=MUL)
                    nc.vector.tensor_tensor(
                        out=qs16[:], in0=qs16[:], in1=csd_sb[:, 1, :, ts], op=MUL)
                    nc.vector.tensor_tensor(
                        out=qkT[:, :, ts], in0=qk16[:], in1=qs16[:], op=ADD)

                    # v -> token-major via PE transpose (fp16); Scalar copies
                    vtmp = rtmp.tile([128, 512], f16, tag="vtmp", name="vtmp")
                    nc.scalar.activation(out=vtmp[:], in_=v_ps[:, 0:512], func=COPY)
                    vtr = v_ps[:, 512:1024].bitcast(f16)
                    for s4 in range(4):
                        kt = tt * 4 + s4
                        tr = vtr[:, s4 * 128:(s4 + 1) * 128]
                        nc.tensor.transpose(tr, vtmp[:, s4 * 128:(s4 + 1) * 128], identb)
                        dstv = v_sb[:, kt, :].rearrange("p (u c) -> p u c", u=2)[:, :, 0:64]
                        srcv = tr.rearrange("p (u c) -> p u c", u=2)
                        nc.vector.tensor_copy(out=dstv, in_=srcv)

                # denominator ones-columns, once per unit
                nc.vector.tensor_copy(out=v_sb[:, :, 64:65], in_=onesb[:, 0:16])
                nc.vector.tensor_copy(out=v_sb[:, :, 129:130], in_=onesb[:, 0:16])

                # ================= attention (qi descending, 2-deep) =========
                aoT = aop.tile([128, S], f16, tag="aoT", name="aoT")
                deque = []  # (kt, at, dp, outT, n_kt, vsb)

                def emit_attnv(kt, at, dp, outT, n_kt, vsb):
                    for h in (0, 1):
                        lhs = vsb[:, kt, :].rearrange("p (u c) -> p u c", u=2)[:, h, :]
                        if dp < 2:
                            nc.tensor.matmul(
                                outT[0:65, h * 512:(h + 1) * 512],
                                lhs,
                                at[:, h * 512:(h + 1) * 512],
                                start=(kt == 0), stop=(kt == n_kt - 1),
                                skip_group_check=True,
                            )
                        else:
                            nc.tensor.matmul(
                                outT[0:65, h * 512 + 256:h * 512 + 512],
                                lhs,
                                at[:, h * 512 + 256:h * 512 + 512],
                                start=False, stop=(kt == n_kt - 1),
                                skip_group_check=True,
                            )

                for qi in range(4):
                    qsl = slice(qi * 512, (qi + 1) * 512)
                    outT = psp.tile([128, 1024], f32, tag="outT", bufs=2, name="outT")
                    n_kt = 4 * qi + 4

                    for kt in range(n_kt):
                        ksl = slice(kt * 128, (kt + 1) * 128)
                        dp = kt - 4 * qi
                        sc = psp.tile([128, 1024], f32, tag="sc", bufs=2, name="sc")
                        at = attnp.tile([128, 1024], f16, tag="at", name="at")
                        if dp < 2:
                            for h in (0, 1):
                                hp = slice(h * 64, (h + 1) * 64)
                                nc.tensor.matmul(
                                    sc[:, h * 512:(h + 1) * 512],
                                    qkT[hp, 1, ksl],
                                    qkT[hp, 0, qsl],
                                    start=True, stop=True,
                                    skip_group_check=True,
                                )
                            nc.scalar.activation(out=at[:], in_=sc[:], func=EXP,
                                                 scale=0.125)
                            if dp == 0:
                                for h in (0, 1):
                                    o = h * 512
                                    nc.vector.tensor_tensor(
                                        out=at[:, o:o + 128], in0=at[:, o:o + 128],
                                        in1=mask0[:, 0:128], op=MUL)
                            elif dp == 1:
                                for h in (0, 1):
                                    o = h * 512
                                    nc.vector.tensor_tensor(
                                        out=at[:, o:o + 256], in0=at[:, o:o + 256],
                                        in1=mask1[:, 0:256], op=MUL)
                        else:
                            # kt2/kt3 of the diagonal: only q columns 256:512
                            for h in (0, 1):
                                hp = slice(h * 64, (h + 1) * 64)
                                o = h * 512
                                nc.tensor.matmul(
                                    sc[:, o + 256:o + 512],
                                    qkT[hp, 1, ksl],
                                    qkT[hp, 0, qsl][:, 256:512],
                                    start=True, stop=True,
                                    skip_group_check=True,
                                )
                            sc3 = sc.rearrange("p (h q) -> p h q", h=2)
                            at3 = at.rearrange("p (h q) -> p h q", h=2)
                            nc.scalar.activation(
                                out=at3[:, :, 256:512], in_=sc3[:, :, 256:512],
                                func=EXP, scale=0.125)
                            msk = mask0[:, 0:128] if dp == 2 else mask1[:, 0:256]
                            w = 128 if dp == 2 else 256
                            for h in (0, 1):
                                o = h * 512
                                nc.vector.tensor_tensor(
                                    out=at[:, o + 256:o + 256 + w],
                                    in0=at[:, o + 256:o + 256 + w],
                                    in1=msk, op=MUL)

                        if len(deque) >= 2:
                            emit_attnv(*deque.pop(0))
                        # at kt==1 the pop above just emitted the PREVIOUS
                        # slab's final attn@V, so its outT is complete and
                        # can be normalized + shipped.
                        if kt == 1 and pending:
                            emit_normalize()
                        if u == 1 and qi == 0 and kt == 3:
                            # prior unit's second O-projection: chunk c(0,3)
                            # was triggered at our tt0 (~25us ago) and has
                            # landed; no collective has been triggered since,
                            # so no false cumulative wait. The "outT" ring
                            # slot freed by v_ps(tt3) hosts y_ps.
                            o_projection(0, 1)
                        deque.append((kt, at, dp, outT, n_kt, v_sb))
                    # the last slab of each unit pends for normalize later
                    pending.append((u, qi, outT, aoT))

                # flush the last two attn@V before the next unit's
                # projections (their outT is consumed at the next
                # emit_normalize)
                while deque:
                    emit_attnv(*deque.pop(0))

            # tail: gather everything already landed BEFORE the c(1,3)
            # trigger (skipping the cumulative cc wait), ship the last
            # chunk, then run o_proj(1,0) UNDER the final AllToAll, and
            # finish with o_proj(1,1) which genuinely waits c(1,3).
            g10 = o_proj_gather(B - 1, 0, tag="g0")
            # separate POOL: all writes into one pool share one DMA-completion
            # semaphore, so a same-pool g11 would fold its c(1,3) wait into
            # o_proj(1,0)'s weight-load threshold
            g11 = gp2.tile([128, 8, 128], f16, tag="g1", name="g")
            nc.sync.dma_start(
                g11[:, :, 0:64], a2aout[B - 1][2].rearrange("j p t -> p j t"))
            emit_normalize(final=True)
            o_proj_compute(B - 1, 0, g10)
            # scalar queue: a sync-queue DMA here would fold its c(1,3) wait
            # into the cumulative sync-DMA semaphore that o_proj(1,0)'s
            # weight loads above check, serializing them behind the last
            # collective
            nc.scalar.dma_start(
                g11[:, :, 64:128], a2aout[B - 1][3].rearrange("j p t -> p j t"))
            o_proj_compute(B - 1, 1, g11)

    nc.compile()
    return nc


def _host_inputs(x, wq, wk, wv, wo):
    # x: [B, S, D] -> xh[u, tt, p, dc, t] = x[u, tt*512 + t, dc*128 + p]
    xh = np.ascontiguousarray(
        x.reshape(B, 4, 512, 8, 128).transpose(0, 1, 4, 3, 2)
    ).astype(np.float16)

    p = np.arange(128)
    invf = THETA ** (-2.0 * ((p % 64) // 2) / 64.0)
    ang = invf[:, None] * np.arange(S)[None, :]
    cost = np.cos(ang)
    sinmt = np.sin(ang) * np.where(p % 2 == 0, -1.0, 1.0)[:, None]
    # duplicate along the q|k axis: [128, 2 (cos|sinm), 2 (q|k), S]
    csd = np.stack([cost, sinmt], axis=1)[:, :, None, :].repeat(2, axis=2)
    csd = np.ascontiguousarray(csd).astype(np.float16)

    i = np.arange(128)[:, None]
    j = np.arange(256)[None, :]
    cb = np.zeros((128, 672), np.float16)
    cb[:, 0:256] = (j >= i).astype(np.float16)          # mask0
    cb[:, 256:512] = (j >= i + 128).astype(np.float16)  # mask1
    cb[:, 512:640] = np.eye(128, dtype=np.float16)      # ident
    cb[:, 640:656] = 1.0                                # ones

    woh = np.ascontiguousarray(
        wo.T.reshape(8, 128, D).transpose(1, 0, 2)
    ).astype(np.float16)  # [p, dc, o]

    in_maps = []
    for c in range(NC):
        sl = slice(c * 128, (c + 1) * 128)
        wqkv = np.ascontiguousarray(
            np.stack([wq[sl], wk[sl], wv[sl]]).reshape(3, 128, 8, 128).transpose(3, 0, 2, 1)
        ).astype(np.float16)  # [p, iw, dc, o]
        in_maps.append({
            "xh": xh,
            "wqkv": wqkv,
            "wo": woh,
            "csd": csd,
            "cb": cb,
            "onesr": np.ones((1, 64), np.float32),
        })
    return in_maps


def kernel(x, wq, wk, wv, wo, _trace=False):
    from concourse.bass_utils import run_bass_kernel_spmd

    if "nc" not in _COMPILED:
        _COMPILED["nc"] = _build()
    nc = _COMPILED["nc"]

    in_maps = _host_inputs(
        np.asarray(x, np.float32), np.asarray(wq, np.float32),
        np.asarray(wk, np.float32), np.asarray(wv, np.float32),
        np.asarray(wo, np.float32),
    )
    res = run_bass_kernel_spmd(nc, in_maps, core_ids=list(range(NC)), trace=_trace)
    _COMPILED["last_result"] = res

    y = np.zeros((B, S, D), np.float32)
    for c in range(NC):
        yc = res.results[c]["y"].reshape(B, 4, 64, D)  # [u, qi, j, o]
        for qi in range(4):
            y[:, qi * 512 + c * 64: qi * 512 + (c + 1) * 64, :] = yc[:, qi]
    return y


# revision 41
# speedup vs baseline: 1.0286x; 1.0286x over previous
"""Multi-head self-attention (RoPE, causal) Bass kernel for 8 TRN2 NeuronCores.

Sharding: tensor-parallel over heads for QKV+attention (2 heads/core),
chunked AllToAll (1 chunk per 512-token q-slab, fp16) overlapped with
attention compute, then token-parallel O-projection (strided 64-token
strips per core).

v2 schedule:
  - attention q-slabs processed in DESCENDING order (qi=3..0) so the
    last-shipped AllToAll chunk is the cheapest slab and the tail is
    normalize + one small collective + one O-projection.
  - attn@V matmuls deferred 2 kt-blocks behind the score matmuls
    (global deque across qi boundaries) so the exp->mask chain never
    stalls the PE.
  - RoPE computed in fp16: Scalar casts PSUM->SBUF, DVE does
    shuffle + 3 wide TTs (2x 16-bit mode).
  - v-transpose copies + casts on the Scalar engine (idle in proj phase).
  - O-projections emitted before the next collective trigger so the
    cumulative CC semaphore never adds false waits.
  - final O-projection os-split with early per-half y DMA.

Layouts (per core):
  qkT:   [128 part = 2 heads x 64 dk, 2 (q|k), t] fp16
  sc:    [128 part = k-tile, 2 heads x 512 q] PSUM f32
  at:    same shape, fp16, exp(sc*0.125), causal-masked via 0/1 TT mult
  v_sb:  [128 part = k-tile tokens, 16 kt, 130] fp16 ([v_h0 | 1 | v_h1 | 1])
  outT:  [65 part = 64 dk + denom row, 2 x 512 q] PSUM f32 (both heads)
  aoT:   [128, t] fp16 normalized attention output
  y:     [128 part = 2 x 64-token strips, 1024] f32 per (unit, pair)
"""

import numpy as np

B, S, D, H, DK = 2, 2048, 1024, 16, 64
NC = 8
THETA = 10000.0

_COMPILED = {}


def _build():
    import concourse.bass as bass
    import concourse.tile as tile
    from concourse import bacc, mybir

    f32 = mybir.dt.float32
    f32r = mybir.dt.float32r
    f16 = mybir.dt.float16
    MUL = mybir.AluOpType.mult
    ADD = mybir.AluOpType.add
    EXP = mybir.ActivationFunctionType.Exp
    COPY = mybir.ActivationFunctionType.Copy

    nc = bacc.Bacc(num_devices=NC)

    xh_d = nc.dram_tensor("xh", [B, 4, 128, 8, 512], f16, kind="ExternalInput")
    wqkv_d = nc.dram_tensor("wqkv", [128, 3, 8, 128], f16, kind="ExternalInput")
    wo_d = nc.dram_tensor("wo", [128, 8, 1024], f16, kind="ExternalInput")
    # csd: [128, 2 (cos|sinm), 2 (q|k dup), S] fp16
    csd_d = nc.dram_tensor("csd", [128, 2, 2, S], f16, kind="ExternalInput")
    cb_d = nc.dram_tensor("cb", [128, 672], f16, kind="ExternalInput")
    onesr_d = nc.dram_tensor("onesr", [1, 64], f32r, kind="ExternalInput")
    y_d = nc.dram_tensor("y", [B, 2, 128, 1024], f32, kind="ExternalOutput")

    SWAP_MASK = [(i ^ 1) for i in range(32)]

    with tile.TileContext(nc) as tc:
        with (
            tc.tile_pool(name="const", bufs=1) as constp,
            tc.tile_pool(name="xtp", bufs=2) as xtp,
            tc.tile_pool(name="qk", bufs=2) as qkp,
            tc.tile_pool(name="vp", bufs=2) as vp,
            tc.tile_pool(name="attn", bufs=3) as attnp,
            tc.tile_pool(name="ao", bufs=2) as aop,
            tc.tile_pool(name="rtmp", bufs=2) as rtmp,
            tc.tile_pool(name="recipp", bufs=2) as recipp,
            tc.tile_pool(name="rbp", bufs=2) as rbp,
            tc.tile_pool(name="gp", bufs=2) as gp,
            tc.tile_pool(name="yp", bufs=2) as yp,
            tc.tile_pool(name="ps", bufs=1, space="PSUM") as psp,
            tc.tile_pool(name="dram", bufs=1, space="DRAM") as dramp,
        ):
            # ---- constant tiles ----
            wqkv_sb = constp.tile([128, 3, 8, 128], f16)
            wo_sb = constp.tile([128, 8, 1024], f16)
            csd_sb = constp.tile([128, 2, 2, S], f16)
            cb_sb = constp.tile([128, 672], f16)
            onesr_sb = constp.tile([1, 64], f32r)

            mask0 = cb_sb[:, 0:256]
            mask1 = cb_sb[:, 256:512]
            identb = cb_sb[:, 512:640]
            onesb = cb_sb[:, 640:656]

            # critical path: projection weights first (sync queue), split so
            # the first dc matmuls can start before the full tensor lands
            nc.sync.dma_start(wqkv_sb[:, :, 0:2, :], wqkv_d[:, :, 0:2, :])
            nc.sync.dma_start(wqkv_sb[:, :, 2:8, :], wqkv_d[:, :, 2:8, :])

            warm_in = dramp.tile([NC, 64], f32, name="warm_in")
            warm_out = dramp.tile([NC, 64], f32, name="warm_out")
            nc.gpsimd.collective_compute(
                "AllToAll",
                mybir.AluOpType.bypass,
                replica_groups=[list(range(NC))],
                ins=[warm_in.opt()],
                outs=[warm_out.opt()],
            )
            # startup DMA bandwidth is the projection-phase bottleneck:
            # load only what the first tiles need, defer the rest
            # (csd half 2 at u0-tt1, wo at u0-tt3)
            nc.gpsimd.dma_start(cb_sb[:], cb_d[:])
            nc.gpsimd.dma_start(onesr_sb[:], onesr_d[:])
            nc.gpsimd.dma_start(csd_sb[:, :, :, 0:1024], csd_d[:, :, :, 0:1024])

            a2ain = [
                [dramp.tile([NC, 128, 64], f16, name=f"a2ai{u}_{c}") for c in range(4)]
                for u in range(B)
            ]
            a2aout = [
                [dramp.tile([NC, 128, 64], f16, name=f"a2ao{u}_{c}") for c in range(4)]
                for u in range(B)
            ]

            def o_proj_gather(u, pair):
                g = gp.tile([128, 8, 128], f16, tag="g", name="g")
                for half in range(2):
                    nc.sync.dma_start(
                        g[:, :, half * 64:(half + 1) * 64],
                        a2aout[u][pair * 2 + half].rearrange("j p t -> p j t"),
                    )
                return g

            def o_proj_compute(u, pair, g, y_tag="outT"):
                y_ps = psp.tile([128, 1024], f32, tag=y_tag, bufs=2, name="y_ps")
                y_sb = yp.tile([128, 1024], f32, tag="y", name="y_sb")
                for os_ in range(2):
                    osl = slice(os_ * 512, (os_ + 1) * 512)
                    for src in range(8):
                        nc.tensor.matmul(
                            y_ps[:, osl],
                            g[:, src, :],
                            wo_sb[:, src, osl],
                            start=(src == 0), stop=(src == 7),
                        )
                    nc.vector.tensor_copy(out=y_sb[:, osl], in_=y_ps[:, osl])
                    nc.scalar.dma_start(y_d[u, pair][:, osl], y_sb[:, osl])

            def o_projection(u, pair, y_tag="outT"):
                o_proj_compute(u, pair, o_proj_gather(u, pair), y_tag)

            # deferred normalize+ship state: (u, qi, outT, aoT)
            pending = []

            def emit_normalize(final=False):
                u, qi, outT, aoT = pending.pop()
                qsl = slice(qi * 512, (qi + 1) * 512)
                dentr = recipp.tile([1, 1024], f32r, tag="recip", name="dentr")
                nc.vector.tensor_copy(out=dentr[:], in_=outT[64:65, :])
                rb_ps = psp.tile([128, 1024], f32, tag="sc", bufs=2, name="rb_ps")
                for h in (0, 1):
                    # broadcast denominators to 64 partitions via an
                    # f32r ones-column outer product
                    nc.tensor.matmul(
                        rb_ps[0:64, h * 512:(h + 1) * 512],
                        onesr_sb[:],
                        dentr[0:1, h * 512:(h + 1) * 512],
                        start=True, stop=True,
                    )
                rb_sb = rbp.tile([128, 1024], f32, tag="rb", name="rb_sb")
                nc.vector.reciprocal_approx_fast(out=rb_sb[0:64, :], in_=rb_ps[0:64, :])
                for h in (0, 1):
                    nc.vector.tensor_tensor(
                        out=aoT[h * 64:(h + 1) * 64, qsl],
                        in0=outT[0:64, h * 512:(h + 1) * 512],
                        in1=rb_sb[0:64, h * 512:(h + 1) * 512],
                        op=MUL,
                    )
                # ship this q-slab as its own a2a chunk
                dst = a2ain[u][qi].rearrange("j p t -> p j t")
                src = aoT[:, qsl].rearrange("p (j t) -> p j t", j=8)
                nc.gpsimd.dma_start(dst, src)
                nc.gpsimd.collective_compute(
                    "AllToAll",
                    mybir.AluOpType.bypass,
                    replica_groups=[list(range(NC))],
                    ins=[a2ain[u][qi].opt()],
                    outs=[a2aout[u][qi].opt()],
                )

            for u in range(B):
                # ================= projections + RoPE =================
                qkT = qkp.tile([128, 2, S], f16, tag="qkT", name="qkT")
                v_sb = vp.tile([128, 16, 130], f16, tag="v", name="v_sb")

                for tt in range(4):
                    ts = slice(tt * 512, (tt + 1) * 512)
                    xt_sb = xtp.tile([128, 8, 512], f16, tag="xt", name="xt_sb")
                    if u == 0 and tt == 0:
                        nc.sync.dma_start(xt_sb[:, 0:2, :], xh_d[u, tt][:, 0:2, :])
                        nc.sync.dma_start(xt_sb[:, 2:8, :], xh_d[u, tt][:, 2:8, :])
                    else:
                        nc.sync.dma_start(xt_sb[:], xh_d[u, tt])
                    if u == 0 and tt == 1:
                        nc.gpsimd.dma_start(csd_sb[:, :, :, 1024:2048],
                                            csd_d[:, :, :, 1024:2048])
                    if u == 0 and tt == 3:
                        nc.gpsimd.dma_start(wo_sb[:], wo_d[:])
                    qk_ps = psp.tile([128, 1024], f32, tag="sc", bufs=2, name="qk_ps")
                    v_ps = psp.tile([128, 1024], f32, tag="outT", bufs=2, name="v_ps")
                    for dc in range(8):
                        st = dc == 0
                        sp = dc == 7
                        rhs = xt_sb[:, dc, :]
                        nc.tensor.matmul(qk_ps[:, 0:512], wqkv_sb[:, 0, dc, :], rhs, start=st, stop=sp)
                        nc.tensor.matmul(qk_ps[:, 512:1024], wqkv_sb[:, 1, dc, :], rhs, start=st, stop=sp)
                        nc.tensor.matmul(v_ps[:, 0:512], wqkv_sb[:, 2, dc, :], rhs, start=st, stop=sp)

                    if u > 0 and tt == 0:
                        # prior unit's first O-projection (chunks 0,1 landed
                        # long ago; must precede the c(u-1,3) trigger so the
                        # cumulative cc semaphore adds no false wait), then
                        # ship the prior unit's last chunk. y_ps on the "sc"
                        # ring here: the "outT" ring still holds the prior
                        # unit's un-normalized outT (deadlock otherwise).
                        o_projection(u - 1, 0, y_tag="sc")
                        while pending:
                            emit_normalize()

                    # RoPE in fp16: Scalar casts, DVE rotates
                    qk16 = rtmp.tile([128, 2, 512], f16, tag="qk16", name="qk16")
                    qs16 = rtmp.tile([128, 2, 512], f16, tag="qs16", name="qs16")
                    nc.scalar.activation(
                        out=qk16.rearrange("p a t -> p (a t)"),
                        in_=qk_ps[:], func=COPY)
                    nc.vector.stream_shuffle(
                        qs16.rearrange("p a t -> p (a t)"),
                        qk16.rearrange("p a t -> p (a t)"), SWAP_MASK)
                    nc.vector.tensor_tensor(
                        out=qk16[:], in0=qk16[:], in1=csd_sb[:, 0, :, ts], op=MUL)
                    nc.vector.tensor_tensor(
                        out=qs16[:], in0=qs16[:], in1=csd_sb[:, 1, :, ts], op=MUL)
                    nc.vector.tensor_tensor(
                        out=qkT[:, :, ts], in0=qk16[:], in1=qs16[:], op=ADD)

                    # v -> token-major via PE transpose (fp16); Scalar copies
                    vtmp = rtmp.tile([128, 512], f16, tag="vtmp", name="vtmp")
                    nc.scalar.activation(out=vtmp[:], in_=v_ps[:, 0:512], func=COPY)
                    vtr = v_ps[:, 512:1024].bitcast(f16)
                    for s4 in range(4):
                        kt = tt * 4 + s4
                        tr = vtr[:, s4 * 128:(s4 + 1) * 128]
                        nc.tensor.transpose(tr, vtmp[:, s4 * 128:(s4 + 1) * 128], identb)
                        dstv = v_sb[:, kt, :].rearrange("p (u c) -> p u c", u=2)[:, :, 0:64]
                        srcv = tr.rearrange("p (u c) -> p u c", u=2)
                        nc.vector.tensor_copy(out=dstv, in_=srcv)

                # denominator ones-columns, once per unit
                nc.vector.tensor_copy(out=v_sb[:, :, 64:65], in_=onesb[:, 0:16])
                nc.vector.tensor_copy(out=v_sb[:, :, 129:130], in_=onesb[:, 0:16])

                # ================= attention (qi descending, 2-deep) =========
                aoT = aop.tile([128, S], f16, tag="aoT", name="aoT")
                deque = []  # (kt, at, dp, outT, n_kt, vsb)

                def emit_attnv(kt, at, dp, outT, n_kt, vsb):
                    for h in (0, 1):
                        lhs = vsb[:, kt, :].rearrange("p (u c) -> p u c", u=2)[:, h, :]
                        if dp < 2:
                            nc.tensor.matmul(
                                outT[0:65, h * 512:(h + 1) * 512],
                                lhs,
                                at[:, h * 512:(h + 1) * 512],
                                start=(kt == 0), stop=(kt == n_kt - 1),
                                skip_group_check=True,
                            )
                        else:
                            nc.tensor.matmul(
                                outT[0:65, h * 512 + 256:h * 512 + 512],
                                lhs,
                                at[:, h * 512 + 256:h * 512 + 512],
                                start=False, stop=(kt == n_kt - 1),
                                skip_group_check=True,
                            )

                for qi in range(4):
                    qsl = slice(qi * 512, (qi + 1) * 512)
                    outT = psp.tile([128, 1024], f32, tag="outT", bufs=2, name="outT")
                    n_kt = 4 * qi + 4

                    for kt in range(n_kt):
                        ksl = slice(kt * 128, (kt + 1) * 128)
                        dp = kt - 4 * qi
                        sc = psp.tile([128, 1024], f32, tag="sc", bufs=2, name="sc")
                        at = attnp.tile([128, 1024], f16, tag="at", name="at")
                        if dp < 2:
                            for h in (0, 1):
                                hp = slice(h * 64, (h + 1) * 64)
                                nc.tensor.matmul(
                                    sc[:, h * 512:(h + 1) * 512],
                                    qkT[hp, 1, ksl],
                                    qkT[hp, 0, qsl],
                                    start=True, stop=True,
                                    skip_group_check=True,
                                )
                            nc.scalar.activation(out=at[:], in_=sc[:], func=EXP,
                                                 scale=0.125)
                            if dp == 0:
                                for h in (0, 1):
                                    o = h * 512
                                    nc.vector.tensor_tensor(
                                        out=at[:, o:o + 128], in0=at[:, o:o + 128],
                                        in1=mask0[:, 0:128], op=MUL)
                            elif dp == 1:
                                for h in (0, 1):
                                    o = h * 512
                                    nc.vector.tensor_tensor(
                                        out=at[:, o:o + 256], in0=at[:, o:o + 256],
                                        in1=mask1[:, 0:256], op=MUL)
                        else:
                            # kt2/kt3 of the diagonal: only q columns 256:512
                            for h in (0, 1):
                                hp = slice(h * 64, (h + 1) * 64)
                                o = h * 512
                                nc.tensor.matmul(
                                    sc[:, o + 256:o + 512],
                                    qkT[hp, 1, ksl],
                                    qkT[hp, 0, qsl][:, 256:512],
                                    start=True, stop=True,
                                    skip_group_check=True,
                                )
                            sc3 = sc.rearrange("p (h q) -> p h q", h=2)
                            at3 = at.rearrange("p (h q) -> p h q", h=2)
                            nc.scalar.activation(
                                out=at3[:, :, 256:512], in_=sc3[:, :, 256:512],
                                func=EXP, scale=0.125)
                            msk = mask0[:, 0:128] if dp == 2 else mask1[:, 0:256]
                            w = 128 if dp == 2 else 256
                            for h in (0, 1):
                                o = h * 512
                                nc.vector.tensor_tensor(
                                    out=at[:, o + 256:o + 256 + w],
                                    in0=at[:, o + 256:o + 256 + w],
                                    in1=msk, op=MUL)

                        if len(deque) >= 2:
                            emit_attnv(*deque.pop(0))
                        # at kt==1 the pop above just emitted the PREVIOUS
                        # slab's final attn@V, so its outT is complete and
                        # can be normalized + shipped.
                        if kt == 1 and pending:
                            emit_normalize()
                        if u == 1 and qi == 0 and kt == 3:
                            # prior unit's second O-projection: chunk c(0,3)
                            # was triggered at our tt0 (~25us ago) and has
                            # landed; no collective has been triggered since,
                            # so no false cumulative wait. The "outT" ring
                            # slot freed by v_ps(tt3) hosts y_ps.
                            o_projection(0, 1)
                        deque.append((kt, at, dp, outT, n_kt, v_sb))
                    # the last slab of each unit pends for normalize later
                    pending.append((u, qi, outT, aoT))

                # flush the last two attn@V before the next unit's
                # projections (their outT is consumed at the next
                # emit_normalize)
                while deque:
                    emit_attnv(*deque.pop(0))

            # tail: gather everything already landed BEFORE the c(1,3)
            # trigger (skipping the cumulative cc wait), ship the last
            # chunk, then run o_proj(1,0) UNDER the final AllToAll, and
            # finish with o_proj(1,1) which genuinely waits c(1,3).
            g10 = o_proj_gather(B - 1, 0)
            g11 = gp.tile([128, 8, 128], f16, tag="g", name="g")
            nc.sync.dma_start(
                g11[:, :, 0:64], a2aout[B - 1][2].rearrange("j p t -> p j t"))
            emit_normalize(final=True)
            o_proj_compute(B - 1, 0, g10)
            nc.sync.dma_start(
                g11[:, :, 64:128], a2aout[B - 1][3].rearrange("j p t -> p j t"))
            o_proj_compute(B - 1, 1, g11)

    nc.compile()
    return nc


def _host_inputs(x, wq, wk, wv, wo):
    # x: [B, S, D] -> xh[u, tt, p, dc, t] = x[u, tt*512 + t, dc*128 + p]
    xh = np.ascontiguousarray(
        x.reshape(B, 4, 512, 8, 128).transpose(0, 1, 4, 3, 2)
    ).astype(np.float16)

    p = np.arange(128)
    invf = THETA ** (-2.0 * ((p % 64) // 2) / 64.0)
    ang = invf[:, None] * np.arange(S)[None, :]
    cost = np.cos(ang)
    sinmt = np.sin(ang) * np.where(p % 2 == 0, -1.0, 1.0)[:, None]
    # duplicate along the q|k axis: [128, 2 (cos|sinm), 2 (q|k), S]
    csd = np.stack([cost, sinmt], axis=1)[:, :, None, :].repeat(2, axis=2)
    csd = np.ascontiguousarray(csd).astype(np.float16)

    i = np.arange(128)[:, None]
    j = np.arange(256)[None, :]
    cb = np.zeros((128, 672), np.float16)
    cb[:, 0:256] = (j >= i).astype(np.float16)          # mask0
    cb[:, 256:512] = (j >= i + 128).astype(np.float16)  # mask1
    cb[:, 512:640] = np.eye(128, dtype=np.float16)      # ident
    cb[:, 640:656] = 1.0                                # ones

    woh = np.ascontiguousarray(
        wo.T.reshape(8, 128, D).transpose(1, 0, 2)
    ).astype(np.float16)  # [p, dc, o]

    in_maps = []
    for c in range(NC):
        sl = slice(c * 128, (c + 1) * 128)
        wqkv = np.ascontiguousarray(
            np.stack([wq[sl], wk[sl], wv[sl]]).reshape(3, 128, 8, 128).transpose(3, 0, 2, 1)
        ).astype(np.float16)  # [p, iw, dc, o]
        in_maps.append({
            "xh": xh,
            "wqkv": wqkv,
            "wo": woh,
            "csd": csd,
            "cb": cb,
            "onesr": np.ones((1, 64), np.float32),
        })
    return in_maps


def kernel(x, wq, wk, wv, wo, _trace=False):
    from concourse.bass_utils import run_bass_kernel_spmd

    if "nc" not in _COMPILED:
        _COMPILED["nc"] = _build()
    nc = _COMPILED["nc"]

    in_maps = _host_inputs(
        np.asarray(x, np.float32), np.asarray(wq, np.float32),
        np.asarray(wk, np.float32), np.asarray(wv, np.float32),
        np.asarray(wo, np.float32),
    )
    res = run_bass_kernel_spmd(nc, in_maps, core_ids=list(range(NC)), trace=_trace)
    _COMPILED["last_result"] = res

    y = np.zeros((B, S, D), np.float32)
    for c in range(NC):
        yc = res.results[c]["y"].reshape(B, 4, 64, D)  # [u, qi, j, o]
        for qi in range(4):
            y[:, qi * 512 + c * 64: qi * 512 + (c + 1) * 64, :] = yc[:, qi]
    return y
